# revision 6
# baseline (speedup 1.0000x reference)
"""Trainium2 Bass kernel for nn_AntisymmetricLayer — v5 (balanced pipeline).

Per 512-token block:
  DMA    : x1,x2 f32->bf16 token-major tiles [128, 4, 128]
  PE     : paired transposing matmuls -> pzs PSUM [d, (t, z|s, 128)]
  ACT    : zst: one FD=1024 copy pzs -> SBUF bf16 [d, (z|s, t, 128)]
           (so z^T = zst[:, :512], s^T = zst[:, 512:] are contiguous)
  PE     : per chunk-pair ph: A_c = P2_c^T z^T, B_c = Q2_c^T s^T into
           2-bank PSUM pair tiles [128, 1024]
  ACT    : bs_ph: FD=1024 copy B-pair -> SBUF bf16
  DVE    : prod_ph = A-pair(PSUM) * bs_ph(SBUF) -> SBUF bf16 [128, 1024]
  PE     : lin matmul (start=True over [128,512] outp bank) then 8 sel
           strip matmuls (32-wide, zero-padded cols, 4 col-groups) accumulate
  DVE    : osb: copy outp -> SBUF f32; DMA 64 used rows -> DRAM [64, n]

out row for k = 8c+t is 32*(c//2) + 8*(c%2) + t; host inverse-permutes.

PSUM: pa pool 2 slots x 2 banks (A-pairs); pb pool 2 slots x 2 banks shared
(tag-rotated) by pzs, 4 B-pairs, outp = 6 allocs/block.
"""

import numpy as np
import ml_dtypes

import concourse.bass as bass
import concourse.mybir as mybir
import concourse.tile as tile
from concourse import bacc
from concourse.bass import ts
from concourse.bass_utils import run_bass_kernel_spmd

F32 = mybir.dt.float32
BF16 = mybir.dt.bfloat16

D = 128
K = 64
R = 16
KR = K * R  # 1024
NCHUNK = KR // 128  # 8
TILE = 128
CT = 4  # token-tiles per block
BLK = TILE * CT  # 512
N_CORES = 8
SELW = NCHUNK * 32  # 256
# idpair|idpairn|p2|q2|wt2A|wt2B|sel2
CONST_W = 2 * KR + 256 + SELW + 2 * 256  # 3072


def build_bass(n_tokens: int = 16384):
    assert n_tokens % BLK == 0
    n_blocks = n_tokens // BLK

    nc = bacc.Bacc(None, target_bir_lowering=False)

    x1 = nc.declare_dram_parameter("x1", [n_tokens, D], F32, isOutput=False)
    x2 = nc.declare_dram_parameter("x2", [n_tokens, D], F32, isOutput=False)
    cw = nc.declare_dram_parameter("cw", [D, CONST_W], BF16, isOutput=False)
    # output stored permuted-transposed [64, n]; host fixes after gather
    out = nc.declare_dram_parameter("out", [K, n_tokens], F32, isOutput=True)
    outv = out.rearrange("(g p) n -> g p n", g=4)

    with tile.TileContext(nc) as tc:
        with (
            tc.tile_pool(name="const", bufs=1) as cpool,
            tc.tile_pool(name="xin", bufs=4) as xpool,
            tc.tile_pool(name="zst", bufs=3) as zpool,
            tc.tile_pool(name="bsp", bufs=6) as bspool,
            tc.tile_pool(name="prods", bufs=6) as ppool,
            tc.tile_pool(name="outs", bufs=4) as opool,
            tc.tile_pool(name="pa", bufs=2, space="PSUM") as pa_pool,
            tc.tile_pool(name="pb", bufs=2, space="PSUM") as pb_pool,
        ):
            cws = cpool.tile([D, CONST_W], BF16)
            nc.sync.dma_start(cws[:, 0:512], cw[:, 0:512])
            nc.sync.dma_start(cws[:, 512:], cw[:, 512:])
            idpair = cws[:, 0:256]
            idpairn = cws[:, 256:512]
            p2s = cws[:, 512 : 512 + KR]
            q2s = cws[:, 512 + KR : 512 + 2 * KR]
            wt2As = cws[:, 512 + 2 * KR : 512 + 2 * KR + 128]
            wt2Bs = cws[:, 512 + 2 * KR + 128 : 512 + 2 * KR + 256]
            sel2s = cws[:, 512 + 2 * KR + 256 : 512 + 2 * KR + 256 + SELW]

            x1v = x1.rearrange("(c a p) d -> c p a d", p=TILE, a=CT)
            x2v = x2.rearrange("(c a p) d -> c p a d", p=TILE, a=CT)

            # PE warm-up: ~2.6us of back-to-back dummy matmuls right after
            # the first const DMA lands, so the HAM clock-gate reaches 8/8
            # before the real pipeline fills.
            warm = pb_pool.tile([D, 2 * BLK], F32, name="warm", tag="pb")
            for _ in range(24):
                nc.tensor.matmul(warm[:, 0:256], idpair[:, 0:128], idpair,
                                 start=True, stop=True)

            xcs = {}
            zsts = {}
            prods = {}
            outps = {}

            def emit_dma(j):
                if j >= n_blocks:
                    return
                x1c = xpool.tile([TILE, CT, D], BF16, name=f"x1c{j}", tag="x1c")
                nc.gpsimd.dma_start(x1c[:], x1v[j])
                x2c = xpool.tile([TILE, CT, D], BF16, name=f"x2c{j}", tag="x2c")
                nc.gpsimd.dma_start(x2c[:], x2v[j])
                xcs[j] = (x1c, x2c)

            def emit_transpose(j):
                # pzs layout [d, (t, zs, 128)]; zst layout [d, (zs, t, 128)]
                if j >= n_blocks:
                    return
                x1c, x2c = xcs[j]
                pzs = pb_pool.tile([D, 2 * BLK], F32, name=f"pzs{j}", tag="pb")
                for t in range(CT):
                    pair = pzs[:, ts(t, 256)]
                    nc.tensor.matmul(pair, x1c[:, t, :], idpair,
                                     start=True, stop=False)
                    nc.tensor.matmul(pair, x2c[:, t, :], idpairn,
                                     start=False, stop=True)
                zst = zpool.tile([D, 2 * BLK], BF16, name=f"zst{j}", tag="zst")
                nc.scalar.copy(
                    zst.rearrange("p (c t w) -> p c t w", c=2, t=CT),
                    pzs.rearrange("p (t c w) -> p c t w", t=CT, c=2),
                )
                zsts[j] = zst

            def emit_phase(j, ph):
                zst = zsts[j]
                ztv = zst[:, 0:BLK]
                stv = zst[:, BLK : 2 * BLK]
                pb = pb_pool.tile([D, 2 * BLK], F32, name=f"pb{j}_{ph}", tag="pb")
                pa = pa_pool.tile([D, 2 * BLK], F32, name=f"pa{j}_{ph}", tag="pa")
                for h in range(2):
                    nc.tensor.matmul(
                        pb[:, ts(h, BLK)], q2s[:, ts(2 * ph + h, 128)], stv,
                        start=True, stop=True,
                    )
                for h in range(2):
                    nc.tensor.matmul(
                        pa[:, ts(h, BLK)], p2s[:, ts(2 * ph + h, 128)], ztv,
                        start=True, stop=True,
                    )
                bs = bspool.tile([D, 2 * BLK], BF16, name=f"bs{j}_{ph}", tag="bs")
                nc.scalar.copy(bs[:], pb[:])
                pr = ppool.tile([D, 2 * BLK], BF16, name=f"pr{j}_{ph}", tag="pr")
                nc.vector.tensor_mul(pr[:], pa[:], bs[:])
                prods.setdefault(j, []).append(pr)

            def emit_sel(j, c):
                g = c // 2
                b = g // 2  # bank: groups 0,1 -> cols 0:512; 2,3 -> 512:1024
                strip = outps[j][32 * g : 32 * g + 32, ts(b, BLK)]
                nc.tensor.matmul(
                    strip,
                    sel2s[:, ts(c, 32)],
                    prods[j][c // 2][:, ts(c % 2, BLK)],
                    start=False,
                    stop=(c in (3, 7)),
                    skip_group_check=True,
                    tile_position=(0, 32 * g),
                )

            def emit_lin(j):
                outp = pa_pool.tile([D, 2 * BLK], F32, name=f"outp{j}", tag="pa")
                outps[j] = outp
                ztv = zsts[j][:, 0:BLK]
                # half-row openers in different banks and col groups: they
                # pack in the PE array alongside the sel strips
                nc.tensor.matmul(outp[0:64, 0:BLK], wt2As[:, 0:64], ztv,
                                 start=True, stop=False, skip_group_check=True,
                                 tile_position=(0, 0))
                nc.tensor.matmul(outp[64:128, BLK : 2 * BLK],
                                 wt2Bs[:, 64:128], ztv,
                                 start=True, stop=False, skip_group_check=True,
                                 tile_position=(0, 64))

            def emit_tail(j):
                # evacuation of block j
                osb = opool.tile([D, 2 * BLK], F32, name=f"osb{j}", tag="osb")
                # single wide evacuation; the never-written PSUM quadrants
                # (rows 64-127 of bank 0, 0-63 of bank 1) are copied but the
                # out-DMAs below read only the initialized strips
                nc.vector.tensor_copy(osb[:], outps[j][:])
                for g in range(4):
                    nc.sync.dma_start(
                        outv[g, :, ts(j, BLK)],
                        osb[32 * g : 32 * g + 16, ts(g // 2, BLK)],
                    )
                del outps[j], prods[j], zsts[j]

            emit_dma(0)
            emit_dma(1)
            emit_transpose(0)
            for j in range(n_blocks):
                emit_dma(j + 2)
                emit_phase(j, 0)
                emit_phase(j, 1)
                emit_transpose(j + 1)
                emit_phase(j, 2)
                emit_phase(j, 3)
                emit_lin(j)
                for c in (0, 4, 2, 6, 1, 5, 3, 7):
                    emit_sel(j, c)
                emit_tail(j)

    nc.finalize()
    return nc


def _perm():
    # out-row for k = 8c+t is 32*(c//2) + 8*(c%2) + t
    perm = np.zeros(K, dtype=np.int64)
    for c in range(NCHUNK):
        for t in range(8):
            perm[8 * c + t] = 32 * (c // 2) + 8 * (c % 2) + t
    return perm


def _make_sel():
    # chunk c stationary cols [32c, 32c+32): col 8*(c%2)+t sums partitions
    # [16t, 16t+16); other cols zero
    sel = np.zeros((128, NCHUNK * 32), dtype=np.float32)
    for c in range(NCHUNK):
        for t in range(8):
            sel[16 * t : 16 * t + 16, 32 * c + 8 * (c % 2) + t] = 1.0
    return sel


def _make_wt2(W_lin):
    # [wt2A | wt2B]: full-width lin stationaries masked to rows 0-63 /
    # 64-127 (banks 0 / 1 of the output accumulator)
    wt2 = np.zeros((D, 128), dtype=np.float32)
    perm = _perm()
    for k in range(K):
        wt2[:, perm[k]] = W_lin[k, :]
    wt2A = wt2.copy()
    wt2A[:, 64:] = 0.0
    wt2B = wt2.copy()
    wt2B[:, :64] = 0.0
    return np.concatenate([wt2A, wt2B], axis=1)


def _shard_and_pack(x1, x2, W_lin, P, Q):
    p2 = P.transpose(1, 0, 2).reshape(D, KR)
    q2 = Q.transpose(1, 0, 2).reshape(D, KR)
    idp = np.eye(D, dtype=np.float32)
    idpair = np.concatenate([idp, idp], axis=1)
    idpairn = np.concatenate([-idp, idp], axis=1)
    cwv = np.concatenate(
        [idpair, idpairn, p2, q2, _make_wt2(W_lin), _make_sel()], axis=1
    ).astype(ml_dtypes.bfloat16)
    assert cwv.shape == (D, CONST_W)

    in_maps = []
    for b in range(N_CORES):
        in_maps.append(
            {
                "x1": np.ascontiguousarray(x1[b]),
                "x2": np.ascontiguousarray(x2[b]),
                "cw": cwv,
            }
        )
    return in_maps


def postprocess(out_raw):
    """Per-core raw DRAM output [64, n] -> [n, K].

    DRAM row for k = 8c+t is 16*(c//2) + 8*(c%2) + t = k (the strided
    evacuation DMA already compacts the 32-row strips), so no permutation.
    """
    return np.ascontiguousarray(out_raw.T)


def kernel(x1, x2, W_lin, P, Q):
    assert x1.shape == (N_CORES, 16384, D) and x2.shape == x1.shape
    nc = build_bass(16384)
    in_maps = _shard_and_pack(x1, x2, W_lin, P, Q)
    res = run_bass_kernel_spmd(nc, in_maps, core_ids=list(range(N_CORES)))
    out = np.stack(
        [postprocess(res.results[b]["out"]) for b in range(N_CORES)], axis=0
    )
    return out.astype(np.float32)
